# revision 3
# baseline (speedup 1.0000x reference)
"""KNN classifier kernel for Trainium2 (8 NeuronCores, Bass/Tile).

Strategy (classic distributed KNN, train-sharded):
  - Shard X_train/y_train along N_train: 12500 rows per core (padded 12544).
  - Per core: score[q, t] = X[q]·T[t] - 0.5*||T[t]||^2  (monotone in -dist)
    computed as float32r matmuls (full-rate fp32 on the PE) with the
    -0.5*||t||^2 term folded in as an extra K=1 accumulation row.
  - DVE hardware top-8 (InstMax + InstMaxIndex) per 2048-wide superblock,
    then a final top-8 over the 56 superblock candidates per query tile.
  - Each core emits [2048, 8] candidate (value, local index) pairs; the
    host all-gathers the 8*8=64 candidates per query, takes the final
    top-8 and majority-votes the labels (tie -> smallest class).
"""

import numpy as np

import concourse.bass as bass
import concourse.mybir as mybir
import concourse.bacc as bacc
import concourse.tile as tile
import concourse.bass_utils as bass_utils

N_TRAIN, D, N_Q, K, NUM_CLASSES = 100000, 512, 2048, 8, 100
N_CORES = 8
NT_SHARD = N_TRAIN // N_CORES          # 12500
NT_PAD = 12544                          # 24*512 + 256, = 98*128
Q_TILES = N_Q // 128                    # 16
# superblocks: (start, width) covering [0, NT_PAD)
SUPERS = [(i * 2048, 2048) for i in range(6)] + [(12288, 256)]
N_SUP = len(SUPERS)                     # 7
CAND = N_SUP * 8                        # 56 candidates per query per core

F32 = mybir.dt.float32
F32R = mybir.dt.float32r
I32 = mybir.dt.int32
U32 = mybir.dt.uint32
AX = mybir.AxisListType
ALU = mybir.AluOpType

_CACHE = {}


def _build():
    nc = bacc.Bacc("TRN2", target_bir_lowering=False, debug=False)

    lhs = nc.dram_tensor("lhs", [D + 1, N_Q], F32R, kind="ExternalInput")     # [X^T; 1]
    rhs = nc.dram_tensor("rhs", [D + 1, NT_PAD], F32R, kind="ExternalInput")  # [T^T; -t2/2]
    oval = nc.dram_tensor("cand_val", [N_Q, 8], F32, kind="ExternalOutput")
    oidx = nc.dram_tensor("cand_idx", [N_Q, 8], I32, kind="ExternalOutput")

    with tile.TileContext(nc) as tc:
        with (
            tc.tile_pool(name="lhsp", bufs=1) as lhsp,
            tc.tile_pool(name="rhsp", bufs=2) as rhsp,
            tc.tile_pool(name="scorep", bufs=3) as scorep,
            tc.tile_pool(name="psump", bufs=6, space="PSUM") as psump,
            tc.tile_pool(name="candp", bufs=1) as candp,
            tc.tile_pool(name="smallp", bufs=8) as smallp,
            tc.tile_pool(name="constp", bufs=1) as constp,
        ):
            # --- resident tiles ---
            lhs_sb = []
            for dk in range(4):
                t = lhsp.tile([128, N_Q], F32R, tag=f"lhs{dk}")
                nc.sync.dma_start(t[:], lhs[dk * 128:(dk + 1) * 128, :])
                lhs_sb.append(t)
            ones_sb = constp.tile([1, 128], F32R, tag="ones")
            nc.sync.dma_start(ones_sb[0:1, :], lhs[D:D + 1, 0:128])

            cand_val = candp.tile([128, Q_TILES * CAND], F32, tag="cval")
            cand_idx = candp.tile([128, Q_TILES * CAND], F32, tag="cidx")

            iota_i = constp.tile([128, CAND], I32, tag="iotai")
            nc.gpsimd.iota(iota_i[:], pattern=[[1, CAND]], base=0,
                           channel_multiplier=0)
            iota_f = constp.tile([128, CAND], F32, tag="iotaf")
            nc.vector.tensor_copy(iota_f[:], iota_i[:])

            # --- phase 1: scores + per-superblock top-8 ---
            for si, (t0, w) in enumerate(SUPERS):
                rhs_sb = []
                for dk in range(4):
                    t = rhsp.tile([128, w], F32R, tag=f"rhs{dk}")
                    nc.sync.dma_start(t[:, 0:w],
                                      rhs[dk * 128:(dk + 1) * 128, t0:t0 + w])
                    rhs_sb.append(t)
                t2_sb = rhsp.tile([1, w], F32R, tag="t2")
                nc.sync.dma_start(t2_sb[0:1, 0:w], rhs[D:D + 1, t0:t0 + w])

                for qt in range(Q_TILES):
                    scores = scorep.tile([128, w], F32, tag="scores")
                    for c0 in range(0, w, 512):
                        cw = min(512, w - c0)
                        ps = psump.tile([128, cw], F32, tag="ps")
                        for dk in range(4):
                            nc.tensor.matmul(
                                ps[:, 0:cw],
                                lhs_sb[dk][:, qt * 128:(qt + 1) * 128],
                                rhs_sb[dk][:, c0:c0 + cw],
                                start=(dk == 0), stop=False)
                        nc.tensor.matmul(
                            ps[:, 0:cw], ones_sb[0:1, :],
                            t2_sb[0:1, c0:c0 + cw],
                            start=False, stop=True)
                        nc.scalar.copy(scores[:, c0:c0 + cw], ps[:, 0:cw])

                    col = qt * CAND + si * 8
                    vslice = cand_val[:, col:col + 8]
                    nc.vector.max(vslice, scores[:, 0:w])
                    spos = smallp.tile([128, 8], U32, tag="spos")
                    nc.vector.max_index(spos[:], vslice, scores[:, 0:w])
                    sposf = smallp.tile([128, 8], F32, tag="sposf")
                    nc.vector.tensor_copy(sposf[:], spos[:])
                    nc.vector.tensor_scalar_add(cand_idx[:, col:col + 8],
                                                sposf[:], float(t0))

            # --- phase 2: per-query-tile merge of the 56 candidates ---
            for qt in range(Q_TILES):
                blk_v = cand_val[:, qt * CAND:(qt + 1) * CAND]
                blk_i = cand_idx[:, qt * CAND:(qt + 1) * CAND]
                fval = smallp.tile([128, 8], F32, tag="fval")
                nc.vector.max(fval[:], blk_v)
                fpos = smallp.tile([128, 8], U32, tag="fpos")
                nc.vector.max_index(fpos[:], fval[:], blk_v)
                fposf = smallp.tile([128, 8], F32, tag="fposf")
                nc.vector.tensor_copy(fposf[:], fpos[:])

                gidxf = smallp.tile([128, 8], F32, tag="gidxf")
                eq = smallp.tile([128, CAND], F32, tag="eq")
                for j in range(8):
                    nc.vector.tensor_scalar(eq[:], iota_f[:],
                                            fposf[:, j:j + 1], None,
                                            op0=ALU.is_equal)
                    nc.vector.tensor_mul(eq[:], eq[:], blk_i)
                    nc.vector.reduce_sum(gidxf[:, j:j + 1], eq[:], axis=AX.X)
                gidx_i = smallp.tile([128, 8], I32, tag="gidxi")
                nc.vector.tensor_copy(gidx_i[:], gidxf[:])

                nc.sync.dma_start(oval[qt * 128:(qt + 1) * 128, :], fval[:])
                nc.sync.dma_start(oidx[qt * 128:(qt + 1) * 128, :], gidx_i[:])

    nc.compile()
    return nc


def _prep_inputs(X, X_train):
    X = np.ascontiguousarray(np.asarray(X, dtype=np.float32))
    X_train = np.ascontiguousarray(np.asarray(X_train, dtype=np.float32))
    lhs = np.ones((D + 1, N_Q), dtype=np.float32)         # row 512 = ones
    lhs[0:D] = X.T
    in_maps = []
    for c in range(N_CORES):
        shard = X_train[c * NT_SHARD:(c + 1) * NT_SHARD]  # [12500, 512]
        t2 = np.einsum("td,td->t", shard, shard, dtype=np.float32)
        rhsm = np.zeros((D + 1, NT_PAD), dtype=np.float32)
        rhsm[0:D, 0:NT_SHARD] = shard.T
        rhsm[D, 0:NT_SHARD] = -0.5 * t2
        rhsm[D, NT_SHARD:] = -1.0e30
        in_maps.append({"lhs": lhs, "rhs": rhsm})
    return in_maps


def _merge_host(results, y_train):
    y_train = np.asarray(y_train)
    vals = np.concatenate([r["cand_val"] for r in results], axis=1)   # [2048, 64]
    gidx = np.concatenate(
        [r["cand_idx"] + c * NT_SHARD for c, r in enumerate(results)], axis=1)
    order = np.argsort(-vals, axis=1, kind="stable")[:, :K]
    top_idx = np.take_along_axis(gidx, order, axis=1)                  # [2048, 8]
    labels = y_train[top_idx]                                          # [2048, 8]
    counts = np.zeros((N_Q, NUM_CLASSES), dtype=np.int32)
    rows = np.repeat(np.arange(N_Q), K)
    np.add.at(counts, (rows, labels.reshape(-1)), 1)
    return counts.argmax(axis=1).astype(y_train.dtype)


def run(X, X_train, y_train, k, trace=False, **trace_kwargs):
    assert int(k) == K
    if "nc" not in _CACHE:
        _CACHE["nc"] = _build()
    nc = _CACHE["nc"]
    in_maps = _prep_inputs(X, X_train)
    res = bass_utils.run_bass_kernel_spmd(
        nc, in_maps, core_ids=list(range(N_CORES)), trace=trace,
        **trace_kwargs)
    y_pred = _merge_host(res.results, y_train)
    return y_pred, res


def kernel(X, X_train, y_train, k):
    y_pred, _ = run(X, X_train, y_train, k)
    return y_pred
